# revision 7
# baseline (speedup 1.0000x reference)
"""DeepseekV3 mini MoE MLP on 8 TRN2 NeuronCores.

Strategy: expert-parallel. The router (tiny: 0.1% of FLOPs) is computed
with jax ops that mirror the reference bit-for-bit; tokens are then
dispatched on the host to per-expert batches (the "all-to-all"), one
expert per NeuronCore. Each core runs a fused gate/up/silu/mul/down
kernel over its routed tokens in f32r (FP22 single-pass matmul — full
TensorE rate, fp32 storage). The combine (scatter-add weighted by the
top-k routing weights) happens on the host; the routing weight itself is
applied on-device.

Layouts are feature-major ([dim, tokens]) so every matmul contracts over
the SBUF partition dim with no transposes anywhere on device.
"""

import numpy as np

import concourse.bass as bass
import concourse.mybir as mybir
import concourse.tile as tile
from concourse import bacc
from concourse.bass_utils import run_bass_kernel_spmd

DIM = 1024
HIDDEN = 1024
NUM_EXPERTS = 8
TOP_K = 2
P = 128
TT = 512  # token tile (PSUM bank = 512 fp32)
DT = DIM // P  # 8 d-tiles
HT = HIDDEN // P  # 8 h-tiles

F32 = mybir.dt.float32
F32R = mybir.dt.float32r

_program_cache: dict[int, object] = {}
LAST_RESULT = None


def _build_expert_program(C: int):
    """One-expert FFN over C tokens: yt = ((silu(x@wg.T) * (x@wu.T)) @ wd.T).T * cw.

    DRAM params (per core):
      xt [DIM, C]    tokens, transposed (d-major)
      wg [DIM, HIDDEN]  gate_proj[e].T
      wu [DIM, HIDDEN]  up_proj[e].T
      wd [HIDDEN, DIM]  down_proj[e].T
      cw [1, C]      per-token routing weight
      yt [DIM, C]    output, transposed
    """
    assert C % TT == 0
    nt = C // TT

    nc = bacc.Bacc(None, target_bir_lowering=False, debug=False)
    xt = nc.declare_dram_parameter("xt", [DIM, C], F32R, isOutput=False)
    wg = nc.declare_dram_parameter("wg", [DIM, HIDDEN], F32R, isOutput=False)
    wu = nc.declare_dram_parameter("wu", [DIM, HIDDEN], F32R, isOutput=False)
    wd = nc.declare_dram_parameter("wd", [HIDDEN, DIM], F32R, isOutput=False)
    cw = nc.declare_dram_parameter("cw", [P, C], F32, isOutput=False)
    yt = nc.declare_dram_parameter("yt", [DIM, C], F32, isOutput=True)

    with tile.TileContext(nc) as tc:
        with (
            tc.tile_pool(name="wpool", bufs=1) as wpool,
            tc.tile_pool(name="xpool", bufs=2) as xpool,
            tc.tile_pool(name="hpool", bufs=2) as hpool,
            tc.tile_pool(name="apool", bufs=3) as apool,
            tc.tile_pool(name="ypool", bufs=3) as ypool,
            tc.tile_pool(name="cpool", bufs=2) as cpool,
            tc.tile_pool(name="pg", bufs=2, space="PSUM") as pgpool,
            tc.tile_pool(name="pu", bufs=2, space="PSUM") as pupool,
            tc.tile_pool(name="py", bufs=2, space="PSUM") as pypool,
        ):
            # Persistent weights in SBUF: [128, DT * free] laid out (d-tile, col)
            wg_sb = wpool.tile([P, DT * HIDDEN], F32R, tag="wg")
            wu_sb = wpool.tile([P, DT * HIDDEN], F32R, tag="wu")
            wd_sb = wpool.tile([P, HT * DIM], F32R, tag="wd")
            nc.sync.dma_start(
                out=wg_sb[:, :].rearrange("p (a h) -> p a h", a=DT),
                in_=wg.ap().rearrange("(a p) h -> p a h", p=P),
            )
            nc.sync.dma_start(
                out=wu_sb[:, :].rearrange("p (a h) -> p a h", a=DT),
                in_=wu.ap().rearrange("(a p) h -> p a h", p=P),
            )
            nc.sync.dma_start(
                out=wd_sb[:, :].rearrange("p (a d) -> p a d", a=HT),
                in_=wd.ap().rearrange("(a p) d -> p a d", p=P),
            )

            for t in range(nt):
                ts = bass.ts(t, TT)
                x_sb = xpool.tile([P, DT * TT], F32R, tag="x")
                nc.sync.dma_start(
                    out=x_sb[:, :].rearrange("p (a t) -> p a t", a=DT),
                    in_=xt.ap()[:, ts].rearrange("(a p) t -> p a t", p=P),
                )
                cw_sb = cpool.tile([P, TT], F32, tag="cw")
                nc.sync.dma_start(out=cw_sb[:, :], in_=cw.ap()[:, ts])

                h_sb = hpool.tile([P, HT * TT], F32R, tag="h")
                for h in range(HT):
                    pg = pgpool.tile([P, TT], F32, tag="pg")
                    pu = pupool.tile([P, TT], F32, tag="pu")
                    for a in range(DT):
                        nc.tensor.matmul(
                            pg[:, :],
                            wg_sb[:, a * HIDDEN + h * P : a * HIDDEN + h * P + P],
                            x_sb[:, a * TT : (a + 1) * TT],
                            start=(a == 0),
                            stop=(a == DT - 1),
                        )
                    for a in range(DT):
                        nc.tensor.matmul(
                            pu[:, :],
                            wu_sb[:, a * HIDDEN + h * P : a * HIDDEN + h * P + P],
                            x_sb[:, a * TT : (a + 1) * TT],
                            start=(a == 0),
                            stop=(a == DT - 1),
                        )
                    act_sb = apool.tile([P, TT], F32, tag="act")
                    nc.scalar.activation(
                        act_sb[:, :], pg[:, :], mybir.ActivationFunctionType.Sigmoid
                    )
                    sil_sb = apool.tile([P, TT], F32, tag="sil")
                    nc.vector.tensor_tensor(
                        sil_sb[:, :], act_sb[:, :], pg[:, :], mybir.AluOpType.mult
                    )
                    nc.vector.tensor_tensor(
                        h_sb[:, h * TT : (h + 1) * TT],
                        sil_sb[:, :],
                        pu[:, :],
                        mybir.AluOpType.mult,
                    )

                for do in range(HT):
                    py = pypool.tile([P, TT], F32, tag="py")
                    for a in range(HT):
                        nc.tensor.matmul(
                            py[:, :],
                            wd_sb[:, a * DIM + do * P : a * DIM + do * P + P],
                            h_sb[:, a * TT : (a + 1) * TT],
                            start=(a == 0),
                            stop=(a == HT - 1),
                        )
                    y_sb = ypool.tile([P, TT], F32, tag="y")
                    nc.vector.tensor_tensor(
                        y_sb[:, :],
                        py[:, :],
                        cw_sb[:, :],
                        mybir.AluOpType.mult,
                    )
                    nc.sync.dma_start(
                        out=yt.ap()[do * P : (do + 1) * P, ts], in_=y_sb[:, :]
                    )
    nc.compile()
    return nc


def _get_program(C: int):
    if C not in _program_cache:
        _program_cache[C] = _build_expert_program(C)
    return _program_cache[C]


def _route(flat: np.ndarray, gate_w: np.ndarray):
    """Mirror the reference router bit-for-bit (jax ops, same backend)."""
    try:
        import jax
        import jax.numpy as jnp

        logits = jnp.asarray(flat) @ jnp.asarray(gate_w).T
        scores = jax.nn.sigmoid(logits)
        top_val, top_idx = jax.lax.top_k(scores, TOP_K)
        top_val = top_val / (top_val.sum(-1, keepdims=True) + 1e-9)
        return np.asarray(top_val), np.asarray(top_idx)
    except Exception:
        # numpy fallback: identical selection semantics (stable descending)
        logits = flat @ gate_w.T
        scores = 1.0 / (1.0 + np.exp(-logits))
        order = np.argsort(-scores, axis=-1, kind="stable")
        top_idx = order[:, :TOP_K].astype(np.int32)
        top_val = np.take_along_axis(scores, top_idx, axis=-1)
        top_val = top_val / (top_val.sum(-1, keepdims=True) + 1e-9)
        return top_val.astype(np.float32), top_idx


def kernel(x, gate_w, gate_proj, up_proj, down_proj):
    x = np.asarray(x)
    bsz, seqlen, dim = x.shape
    flat = np.ascontiguousarray(x.reshape(-1, dim), dtype=np.float32)
    T = flat.shape[0]
    gate_w = np.asarray(gate_w, dtype=np.float32)
    gate_proj = np.asarray(gate_proj, dtype=np.float32)
    up_proj = np.asarray(up_proj, dtype=np.float32)
    down_proj = np.asarray(down_proj, dtype=np.float32)

    top_val, top_idx = _route(flat, gate_w)

    idx_list = []
    cw_list = []
    for e in range(NUM_EXPERTS):
        mask = top_idx == e  # [T, K]
        tok = np.nonzero(mask.any(axis=1))[0]
        # weight per selected token: sum over k of top_val where top_idx==e
        w = (top_val * mask).sum(axis=1)[tok].astype(np.float32)
        idx_list.append(tok)
        cw_list.append(w)

    max_cnt = max(len(i) for i in idx_list)
    C = ((max_cnt + TT - 1) // TT) * TT
    C = max(C, TT)
    nc = _get_program(C)

    in_maps = []
    for e in range(NUM_EXPERTS):
        tok = idx_list[e]
        cnt = len(tok)
        xt = np.zeros((DIM, C), dtype=np.float32)
        xt[:, :cnt] = flat[tok].T
        cwp = np.zeros((P, C), dtype=np.float32)
        cwp[:, :cnt] = cw_list[e][None, :]
        in_maps.append(
            {
                "xt": xt,
                "wg": np.ascontiguousarray(gate_proj[e].T),
                "wu": np.ascontiguousarray(up_proj[e].T),
                "wd": np.ascontiguousarray(down_proj[e].T),
                "cw": cwp,
            }
        )

    res = run_bass_kernel_spmd(nc, in_maps, core_ids=list(range(NUM_EXPERTS)))
    global LAST_RESULT
    LAST_RESULT = res

    out = np.zeros((T, DIM), dtype=np.float32)
    for e in range(NUM_EXPERTS):
        tok = idx_list[e]
        cnt = len(tok)
        if cnt:
            out[tok] += res.results[e]["yt"][:, :cnt].T
    return out.reshape(bsz, seqlen, dim)


# revision 10
# speedup vs baseline: 1.0612x; 1.0612x over previous
"""DeepseekV3 mini MoE MLP on 8 TRN2 NeuronCores.

Strategy: expert-parallel. The router (tiny: 0.1% of FLOPs) is computed
with jax ops that mirror the reference bit-for-bit; tokens are then
dispatched on the host to per-expert batches (the "all-to-all"), one
expert per NeuronCore. Each core runs a fused gate/up/silu/mul/down
kernel over its routed tokens in f32r (FP22 single-pass matmul — full
TensorE rate, fp32 storage). The combine (scatter-add weighted by the
top-k routing weights) happens on the host.

Layouts are feature-major ([dim, tokens]) so every matmul contracts over
the SBUF partition dim with no transposes anywhere on device. Weights
are passed pre-chunked ([HT, P, DT, P]) so each output-column block's
weights arrive in one fully-contiguous DMA, letting the first matmuls
start ~10us into the kernel instead of waiting for the full 12.6MB.
"""

import numpy as np

import concourse.bass as bass
import concourse.mybir as mybir
import concourse.tile as tile
from concourse import bacc
from concourse.bass_utils import run_bass_kernel_spmd

DIM = 1024
HIDDEN = 1024
NUM_EXPERTS = 8
TOP_K = 2
P = 128
TT = 512  # main token tile (PSUM bank = 512 fp32)
DT = DIM // P  # 8 d-tiles
HT = HIDDEN // P  # 8 h-tiles

F32 = mybir.dt.float32
F32R = mybir.dt.float32r

_program_cache: dict[tuple, object] = {}
LAST_RESULT = None


def _build_expert_program(tiles: tuple):
    """One-expert FFN: yt = ((silu(x@wg.T) * (x@wu.T)) @ wd.T).T over C tokens.

    DRAM params (per core):
      xt [DIM, C]            tokens, transposed (d-major)
      wg/wu [HT, P, DT, P]   gate/up proj, chunked: [h-blk, d-in, d-blk, h-in]
      wd [HT, P, HT, P]      down proj, chunked: [dout-blk, h-in, h-blk, dout-in]
      yt [DIM, C]            output, transposed
    """
    C = sum(tiles)
    nc = bacc.Bacc(None, target_bir_lowering=False, debug=False)
    xt = nc.declare_dram_parameter("xt", [DIM, C], F32R, isOutput=False)
    wg = nc.declare_dram_parameter("wg", [HT, P, DT, P], F32R, isOutput=False)
    wu = nc.declare_dram_parameter("wu", [HT, P, DT, P], F32R, isOutput=False)
    wd = nc.declare_dram_parameter("wd", [HT, P, HT, P], F32R, isOutput=False)
    yt = nc.declare_dram_parameter("yt", [DIM, C], F32, isOutput=True)

    with tile.TileContext(nc) as tc:
        with (
            tc.tile_pool(name="wpool", bufs=1) as wpool,
            tc.tile_pool(name="xpool", bufs=2) as xpool,
            tc.tile_pool(name="hpool", bufs=2) as hpool,
            tc.tile_pool(name="apool", bufs=3) as apool,
            tc.tile_pool(name="ypool", bufs=3) as ypool,
            tc.tile_pool(name="pg", bufs=2, space="PSUM") as pgpool,
            tc.tile_pool(name="pu", bufs=2, space="PSUM") as pupool,
            tc.tile_pool(name="py", bufs=2, space="PSUM") as pypool,
        ):
            # Weight chunk tiles: one per output-column block, so matmuls for
            # block k only depend on chunk k's DMA (fast pipeline ramp).
            wg_c, wu_c, wd_c = [], [], []
            for k in range(HT):
                wg_c.append(wpool.tile([P, DT * P], F32R, name=f"wg{k}", tag=f"wg{k}"))
                wu_c.append(wpool.tile([P, DT * P], F32R, name=f"wu{k}", tag=f"wu{k}"))
            for k in range(HT):
                wd_c.append(wpool.tile([P, HT * P], F32R, name=f"wd{k}", tag=f"wd{k}"))

            first = True
            off = 0
            for t, tt in enumerate(tiles):
                ts = bass.ds(off, tt)
                off += tt
                x_sb = xpool.tile([P, DT * TT], F32R, tag="x")
                nc.sync.dma_start(
                    out=x_sb[:, :].rearrange("p (a t) -> p a t", a=DT)[:, :, :tt],
                    in_=xt.ap()[:, ts].rearrange("(a p) t -> p a t", p=P),
                )
                if first:
                    # Weight DMAs issued after the first x tile: each chunk is
                    # contiguous in DRAM; block-k matmuls start as soon as
                    # chunk k lands.
                    for k in range(HT):
                        nc.sync.dma_start(out=wg_c[k][:, :], in_=wg.ap()[k])
                        nc.sync.dma_start(out=wu_c[k][:, :], in_=wu.ap()[k])
                    for k in range(HT):
                        nc.sync.dma_start(out=wd_c[k][:, :], in_=wd.ap()[k])
                    first = False

                h_sb = hpool.tile([P, HT * TT], F32R, tag="h")
                for h in range(HT):
                    pg = pgpool.tile([P, tt], F32, tag="pg")
                    pu = pupool.tile([P, tt], F32, tag="pu")
                    for a in range(DT):
                        nc.tensor.matmul(
                            pg[:, :],
                            wg_c[h][:, a * P : (a + 1) * P],
                            x_sb[:, a * TT : a * TT + tt],
                            start=(a == 0),
                            stop=(a == DT - 1),
                        )
                    for a in range(DT):
                        nc.tensor.matmul(
                            pu[:, :],
                            wu_c[h][:, a * P : (a + 1) * P],
                            x_sb[:, a * TT : a * TT + tt],
                            start=(a == 0),
                            stop=(a == DT - 1),
                        )
                    act_sb = apool.tile([P, TT], F32, tag="act")
                    nc.scalar.activation(
                        act_sb[:, :tt], pg[:, :], mybir.ActivationFunctionType.Sigmoid
                    )
                    sil_sb = apool.tile([P, TT], F32, tag="sil")
                    nc.vector.tensor_tensor(
                        sil_sb[:, :tt], act_sb[:, :tt], pg[:, :], mybir.AluOpType.mult
                    )
                    nc.vector.tensor_tensor(
                        h_sb[:, h * TT : h * TT + tt],
                        sil_sb[:, :tt],
                        pu[:, :],
                        mybir.AluOpType.mult,
                    )

                for do in range(HT):
                    py = pypool.tile([P, tt], F32, tag="py")
                    for a in range(HT):
                        nc.tensor.matmul(
                            py[:, :],
                            wd_c[do][:, a * P : (a + 1) * P],
                            h_sb[:, a * TT : a * TT + tt],
                            start=(a == 0),
                            stop=(a == HT - 1),
                        )
                    y_sb = ypool.tile([P, TT], F32, tag="y")
                    nc.scalar.copy(y_sb[:, :tt], py[:, :])
                    nc.sync.dma_start(
                        out=yt.ap()[do * P : (do + 1) * P, ts], in_=y_sb[:, :tt]
                    )
    nc.compile()
    return nc


def _tiles_for(max_cnt: int) -> tuple:
    """Token tiles covering max_cnt: full 512s plus one final tile (>=256 for
    full-rate f32r, multiple of 64)."""
    full, rem = divmod(max_cnt, TT)
    if rem == 0:
        return (TT,) * max(full, 1)
    rem = max(256, ((rem + 63) // 64) * 64)
    if rem == TT:
        return (TT,) * (full + 1)
    return (TT,) * full + (rem,)


def _get_program(tiles: tuple):
    if tiles not in _program_cache:
        _program_cache[tiles] = _build_expert_program(tiles)
    return _program_cache[tiles]


def _chunk_w(wt: np.ndarray) -> np.ndarray:
    """[K, M] weight (K contracted) -> chunk layout [m_blk, k_in, k_blk, m_in],
    contiguous per m_blk."""
    K, M = wt.shape
    # wt[k, m] with k = kb*P + kp, m = mb*P + mp  ->  out[mb, kp, kb, mp]
    return np.ascontiguousarray(wt.reshape(K // P, P, M // P, P).transpose(2, 1, 0, 3))


def _route(flat: np.ndarray, gate_w: np.ndarray):
    """Mirror the reference router bit-for-bit (jax ops, same backend)."""
    try:
        import jax
        import jax.numpy as jnp

        logits = jnp.asarray(flat) @ jnp.asarray(gate_w).T
        scores = jax.nn.sigmoid(logits)
        top_val, top_idx = jax.lax.top_k(scores, TOP_K)
        top_val = top_val / (top_val.sum(-1, keepdims=True) + 1e-9)
        return np.asarray(top_val), np.asarray(top_idx)
    except Exception:
        # numpy fallback: identical selection semantics (stable descending)
        logits = flat @ gate_w.T
        scores = 1.0 / (1.0 + np.exp(-logits))
        order = np.argsort(-scores, axis=-1, kind="stable")
        top_idx = order[:, :TOP_K].astype(np.int32)
        top_val = np.take_along_axis(scores, top_idx, axis=-1)
        top_val = top_val / (top_val.sum(-1, keepdims=True) + 1e-9)
        return top_val.astype(np.float32), top_idx


def kernel(x, gate_w, gate_proj, up_proj, down_proj):
    x = np.asarray(x)
    bsz, seqlen, dim = x.shape
    flat = np.ascontiguousarray(x.reshape(-1, dim), dtype=np.float32)
    T = flat.shape[0]
    gate_w = np.asarray(gate_w, dtype=np.float32)
    gate_proj = np.asarray(gate_proj, dtype=np.float32)
    up_proj = np.asarray(up_proj, dtype=np.float32)
    down_proj = np.asarray(down_proj, dtype=np.float32)

    top_val, top_idx = _route(flat, gate_w)

    idx_list = []
    cw_list = []
    for e in range(NUM_EXPERTS):
        mask = top_idx == e  # [T, K]
        tok = np.nonzero(mask.any(axis=1))[0]
        w = (top_val * mask).sum(axis=1)[tok].astype(np.float32)
        idx_list.append(tok)
        cw_list.append(w)

    max_cnt = max(len(i) for i in idx_list)
    tiles = _tiles_for(max_cnt)
    C = sum(tiles)
    nc = _get_program(tiles)

    in_maps = []
    for e in range(NUM_EXPERTS):
        tok = idx_list[e]
        cnt = len(tok)
        xt = np.zeros((DIM, C), dtype=np.float32)
        xt[:, :cnt] = flat[tok].T
        in_maps.append(
            {
                "xt": xt,
                "wg": _chunk_w(gate_proj[e].T),
                "wu": _chunk_w(up_proj[e].T),
                "wd": _chunk_w(down_proj[e].T),
            }
        )

    res = run_bass_kernel_spmd(nc, in_maps, core_ids=list(range(NUM_EXPERTS)))
    global LAST_RESULT
    LAST_RESULT = res

    out = np.zeros((T, DIM), dtype=np.float32)
    for e in range(NUM_EXPERTS):
        tok = idx_list[e]
        cnt = len(tok)
        if cnt:
            out[tok] += (res.results[e]["yt"][:, :cnt] * cw_list[e][None, :]).T
    return out.reshape(bsz, seqlen, dim)
